# revision 20
# baseline (speedup 1.0000x reference)
"""Causal self-attention (B=4, T=2048, C=1024, H=16) on 8 TRN2 NeuronCores.

Sharding:
  - QKV + attention: tensor-parallel over heads (2 heads/core, all batches).
  - Output projection: data-parallel over tokens (256 tokens/core/batch),
    connected by one AllToAll per batch (bf16 payload).

v2 layout (all matmul operands bf16, f32 PSUM accumulate):
  - host passes xT = x^T [C, BT] bf16; per-core W_attn q/k/v slices
    transposed [C, 128] bf16; W_proj^T [C, C] bf16 replicated.
  - QKV computes qkvT [dim, tokens] per 512-token block; q^T/k^T persist in
    SBUF [128, BT]; v^T staged per head as vt [64, T] bf16.
  - V^T staged as one [128, T] bf16 tile per batch (both heads stacked);
    XBAR dma-transposes turn each 128-key block into V tiles [keys, 128]
    (head dims side by side); softmax sums accumulate via a separate
    ones-column matmul (1 moving row).
  - scores in S^T layout [keys, queries] per 128-key tile, both heads in one
    [128, 1024] PSUM tile; exp on ScalarE with 1/8 folded into the scale;
    causal = block-range trimming + additive tri-mask on diagonal blocks.
  - P@V flipped: out y_aug [128 queries, 65] PSUM, lhsT = P^T tile,
    rhs = V_aug -> moving dim is 65 (not 512).  Softmax sums ride along as
    column 64 via the V_aug ones column.
  - division: DVE reciprocal of the two sums + per-partition
    tensor_scalar_mul -> ydiv [queries, 128 dims] bf16, DMA'd token-major
    into the A2A send buffer.
  - after the A2A each core XBAR dma-transposes the whole recv buffer
    [2048, 128] -> y^T [128, 2048] in one instruction; projection lhsT
    slices come straight out of it.
"""

import sys

sys.path.insert(0, "/opt/trn_rl_repo")

import numpy as np
import ml_dtypes

import concourse.bass as bass
import concourse.bacc as bacc
import concourse.mybir as mybir
import concourse.tile as tile
from concourse.bass_utils import run_bass_kernel_spmd

N_CORES = 8
B, T, C = 4, 2048, 1024
H, D = 16, 64
HPC = H // N_CORES          # heads per core = 2
BT = B * T                  # 8192 flattened tokens
QB = 512                    # query block
SB = 128                    # key tile
NQB = T // QB               # 4 query blocks per batch
NSB = T // SB               # 16 key tiles per batch
TOKS = BT // N_CORES        # 1024 output tokens per core
TPB = 256                   # tokens per (core, batch)

F32 = mybir.dt.float32
BF16 = mybir.dt.bfloat16
EXP = mybir.ActivationFunctionType.Exp
BF = ml_dtypes.bfloat16

RUN_KWARGS: dict = {}
LAST_RESULTS = None

_PROGRAM = None


def _build_program():
    nc = bacc.Bacc(num_devices=N_CORES)

    xT = nc.declare_dram_parameter("xT", [C, BT], BF16, isOutput=False)
    wq = nc.declare_dram_parameter("wq", [C, 128], BF16, isOutput=False)
    wk = nc.declare_dram_parameter("wk", [C, 128], BF16, isOutput=False)
    wv = nc.declare_dram_parameter("wv", [C, 128], BF16, isOutput=False)
    wp = nc.declare_dram_parameter("wp", [C, C], BF16, isOutput=False)
    ntri = nc.declare_dram_parameter("ntri", [128, 128], F32, isOutput=False)
    ident = nc.declare_dram_parameter("ident", [128, 128], BF16, isOutput=False)
    out_ext = nc.declare_dram_parameter("out", [TOKS, C], F32, isOutput=True)

    # A2A bounce buffers, token-major: row (m*256 + t) = token t of dest
    # core m, 128 cols = this core's head dims.
    sends = [nc.dram_tensor(f"send{b}", [N_CORES * TPB, 128], BF16) for b in range(B)]
    recvs = [nc.dram_tensor(f"recv{b}", [N_CORES * TPB, 128], BF16) for b in range(B)]

    with tile.TileContext(nc) as tc:
        with (
            tc.tile_pool(name="const", bufs=1) as constp,
            tc.tile_pool(name="wgt", bufs=1) as wgtp,
            tc.tile_pool(name="qk", bufs=1) as qkp,
            tc.tile_pool(name="vt", bufs=2) as vtp,
            tc.tile_pool(name="vaug", bufs=4) as vaugp,
            tc.tile_pool(name="xt", bufs=4) as xtp,
            tc.tile_pool(name="pp", bufs=20) as ptp,
            tc.tile_pool(name="ydiv", bufs=4) as ydp,
            tc.tile_pool(name="rc", bufs=8) as rcp,
            tc.tile_pool(name="rvt", bufs=2) as rvtp,
            tc.tile_pool(name="ob", bufs=2) as obp,
            tc.tile_pool(name="ps", bufs=2, space="PSUM") as psp,       # qkv/proj
            tc.tile_pool(name="sps", bufs=2, space="PSUM") as sps,      # S^T [128,1024]
            tc.tile_pool(name="yaps", bufs=2, space="PSUM") as yaps,    # y_aug per head
        ):
            # ---------------- constants + attention weights ----------------
            wq_s = wgtp.tile([128, 8 * 128], BF16, tag="wq")
            wk_s = wgtp.tile([128, 8 * 128], BF16, tag="wk")
            wv_s = wgtp.tile([128, 8 * 128], BF16, tag="wv")
            for dst, src, eng in (
                (wq_s, wq, nc.sync),
                (wk_s, wk, nc.scalar),
                (wv_s, wv, nc.sync),
            ):
                eng.dma_start(
                    out=dst[:].rearrange("p (c d) -> p c d", c=8),
                    in_=src[:].rearrange("(c p) d -> p c d", p=128),
                )
            ntri_s = constp.tile([128, 128], F32, tag="ntri")
            nc.scalar.dma_start(out=ntri_s[:], in_=ntri[:])
            ident_s = constp.tile([128, 128], BF16, tag="ident")
            nc.scalar.dma_start(out=ident_s[:], in_=ident[:])
            # wp_s is loaded lazily (emitted after phase1(0)) so it doesn't
            # delay the first xt loads.
            wp_s = wgtp.tile([128, 8 * 1024], BF16, tag="wp")

            def load_wp():
                nc.sync.dma_start(
                    out=wp_s[:].rearrange("p (c d) -> p c d", c=8),
                    in_=wp[:].rearrange("(c p) d -> p c d", p=128),
                )

            qT = qkp.tile([128, BT], BF16, tag="qT")
            kT = qkp.tile([128, BT], BF16, tag="kT")

            vaug = {}   # (b, h) -> [128, NSB*65] bf16 V_aug (ones col at 64)
            vt_hb = {}  # b -> [128, T] bf16 (v^T, both heads stacked)

            def qkv_gen(b, tb):
                """QKV for one 512-token block (generator: yields between
                small PE chunks so attention can interleave)."""
                if tb == 0:
                    vt_hb[b] = vtp.tile([128, T], BF16, tag="vt", name=f"vt{b}")
                base = b * T + tb * QB
                xt = xtp.tile([128, 8 * QB], BF16, tag="xt")
                xt4 = xt[:].rearrange("p (c t) -> p c t", c=8)
                src4 = xT[:].rearrange("(c p) t -> p c t", p=128)[
                    :, :, tb * QB + b * T : tb * QB + b * T + QB
                ]
                # halves on separate queues: if a collective blocks the
                # Pool queue, the sync half still lands and QKV proceeds
                nc.sync.dma_start(out=xt4[:, 0:4, :], in_=src4[:, 0:4, :])
                nc.gpsimd.dma_start(out=xt4[:, 4:8, :], in_=src4[:, 4:8, :])
                yield
                pq = psp.tile([128, QB], F32, tag="ps")
                pk = psp.tile([128, QB], F32, tag="ps")
                for c in range(8):
                    st = dict(start=(c == 0), stop=(c == 7))
                    xc = xt[:, c * QB : (c + 1) * QB]
                    nc.tensor.matmul(pq[:], wq_s[:, c * 128 : (c + 1) * 128], xc, **st)
                    nc.tensor.matmul(pk[:], wk_s[:, c * 128 : (c + 1) * 128], xc, **st)
                    yield
                nc.vector.tensor_copy(qT[:, base : base + QB], pq[:])
                nc.vector.tensor_copy(kT[:, base : base + QB], pk[:])
                yield
                pv = psp.tile([128, QB], F32, tag="ps")
                for c in range(8):
                    nc.tensor.matmul(
                        pv[:], wv_s[:, c * 128 : (c + 1) * 128], xt[:, c * QB : (c + 1) * QB],
                        start=(c == 0), stop=(c == 7),
                    )
                    if c % 2 == 1:
                        yield
                tloc = tb * QB
                nc.vector.tensor_copy(vt_hb[b][:, tloc : tloc + QB], pv[:])
                yield

            def vaug_gen(b):
                """PE-transpose each 128-key block of v^T [128, T] (both
                heads at once), then per-head DVE copies into V_aug
                [keys, 65-blocks] whose ones column (from the initial
                memset) gives the softmax sums in the same matmul
                accumulation group as the dims."""
                vas = []
                for h in range(HPC):
                    va = vaugp.tile([128, NSB * 65], BF16, tag="vaug", name=f"va{b}_{h}")
                    vaug[(b, h)] = va
                    nc.vector.memset(va[:], 1.0)
                    vas.append(va)
                yield
                for j in range(NSB):
                    tr = psp.tile([128, 128], BF16, tag="ps", name="tr")
                    nc.tensor.transpose(
                        tr[:], vt_hb[b][:, j * SB : (j + 1) * SB], ident_s[:, :]
                    )
                    for h in range(HPC):
                        nc.vector.tensor_copy(
                            vas[h][:, j * 65 : j * 65 + 64],
                            tr[:, h * 64 : h * 64 + 64],
                        )
                    if j % 2 == 1:
                        yield

            pt_store = {}  # (b, i) -> list of pt3 views

            def scores_gen(b, i):
                """Scores + exp for all key tiles of one query block."""
                jmax = 4 * (i + 1)
                pt3s = []
                pt_store[(b, i)] = pt3s
                for j in range(jmax):
                    diag = j >= 4 * i
                    r = SB * j - QB * i if diag else 0
                    sp = sps.tile([128, 2 * QB], F32, tag="sp")
                    sp3 = sp[:].rearrange("p (h t) -> p h t", h=2)
                    for h in range(HPC):
                        nc.tensor.matmul(
                            sp3[:, h, r:QB],
                            kT[h * 64 : (h + 1) * 64, b * T + j * SB : b * T + (j + 1) * SB],
                            qT[h * 64 : (h + 1) * 64, b * T + i * QB + r : b * T + (i + 1) * QB],
                            start=True, stop=True,
                        )
                        if diag:
                            nc.vector.tensor_add(
                                sp3[:, h, r : r + 128], sp3[:, h, r : r + 128], ntri_s[:]
                            )
                    pt = ptp.tile([128, 2 * QB], BF16, tag="P")
                    pt3 = pt[:].rearrange("p (h t) -> p h t", h=2)
                    nc.scalar.activation(pt3[:, :, r:QB], sp3[:, :, r:QB], EXP, scale=0.125)
                    pt3s.append(pt3)
                    yield

            def pv_gen(b, i):
                """P@V + softmax division + A2A-send for one query block.
                One PSUM bank per head: accumulation groups in a bank must be
                strictly sequential (sim zero-region semantics)."""
                pt3s = pt_store[(b, i)]
                yah = [yaps.tile([128, 512], F32, tag="ya", name=f"ya{h}") for h in range(HPC)]
                ydiv = None
                for qc in range(4):
                    pi = qc % 2
                    if pi == 0:
                        ydiv = ydp.tile([128, 2 * 128], BF16, tag="ydiv")
                    njt = 4 * i + qc + 1
                    for h in range(HPC):
                        for j in range(njt):
                            nc.tensor.matmul(
                                yah[h][:, 0:65],
                                pt3s[j][:, h, qc * 128 : (qc + 1) * 128],
                                vaug[(b, h)][:, j * 65 : j * 65 + 65],
                                start=(j == 0), stop=(j == njt - 1),
                            )
                        yield
                    for h in range(HPC):
                        rc = rcp.tile([128, 1], F32, tag="rc")
                        nc.vector.reciprocal(rc[:], yah[h][:, 64:65])
                        nc.vector.tensor_scalar_mul(
                            ydiv[:, pi * 128 + h * 64 : pi * 128 + h * 64 + 64],
                            yah[h][:, 0:64],
                            rc[:],
                        )
                    if pi == 1:
                        m = 2 * i + qc // 2
                        nc.gpsimd.dma_start(
                            out=sends[b][m * TPB : (m + 1) * TPB, :].rearrange(
                                "(x q) c -> q x c", x=2
                            ),
                            in_=ydiv[:].rearrange("p (x c) -> p x c", x=2),
                        )
                        yield
                del pt_store[(b, i)]

            def a2a(b):
                nc.gpsimd.collective_compute(
                    "AllToAll",
                    mybir.AluOpType.bypass,
                    replica_groups=[list(range(N_CORES))],
                    ins=[sends[b][:]],
                    outs=[recvs[b][:]],
                )

            rvt_store = {}

            def rvt_gen(b):
                """One XBAR dma-transpose turns the whole recv buffer into
                y^T [128, 2048] (prefetchable as soon as the A2A lands)."""
                rvT = rvtp.tile([128, N_CORES * TPB], BF16, tag="rvT", name=f"rvT{b}")
                rvt_store[b] = rvT
                nc.sync.dma_start_transpose(rvT[:], recvs[b][:])
                yield

            def projmm_gen(b):
                """Projection matmuls for this core's 256 tokens of batch b."""
                rvT = rvt_store[b]
                for tt in range(2):
                    for co in range(2):
                        pj = psp.tile([128, 512], F32, tag="ps", name="pj")
                        for c in range(8):
                            nc.tensor.matmul(
                                pj[:],
                                rvT[:, c * TPB + tt * 128 : c * TPB + tt * 128 + 128],
                                wp_s[:, c * 1024 + co * 512 : c * 1024 + (co + 1) * 512],
                                start=(c == 0), stop=(c == 7),
                            )
                            if c % 2 == 1:
                                yield
                        ob = obp.tile([128, 512], F32, tag="ob")
                        nc.vector.tensor_copy(ob[:], pj[:])
                        row = b * TPB + tt * 128
                        eng = nc.sync if co == 0 else nc.scalar
                        eng.dma_start(
                            out=out_ext[row : row + 128, co * 512 : (co + 1) * 512],
                            in_=ob[:],
                        )
                        yield

            # ---------------- emission schedule ----------------
            from collections import deque

            class Feeder:
                def __init__(self):
                    self.q = deque()
                    self.added = 0
                    self.finished = 0

                def add(self, *gens):
                    self.q.extend(gens)
                    self.added += len(gens)
                    return self.added

                def feed(self, n=1):
                    done = 0
                    while self.q and done < n:
                        try:
                            next(self.q[0])
                            done += 1
                        except StopIteration:
                            self.q.popleft()
                            self.finished += 1
                    return done

                def drain_to(self, mark):
                    while self.finished < mark and self.q:
                        self.feed(64)

                def drain(self):
                    while self.feed(64):
                        pass

            def phase1_gens(b):
                return [qkv_gen(b, tb) for tb in range(4)] + [vaug_gen(b)]

            feeder = Feeder()

            def drive(gen, per_step=1):
                n = 0
                for _ in gen:
                    feeder.feed(per_step)
                    n += 1
                return n

            def run_pair(pv, sc):
                """Drive one unit's P@V to completion while interleaving the
                next unit's scores (2 score steps per P@V step) and feeder
                fillers -- PE gets P@V work while ACT chews on the exps."""
                while True:
                    try:
                        next(pv)
                    except StopIteration:
                        break
                    for _ in range(2):
                        if sc is not None:
                            try:
                                next(sc)
                            except StopIteration:
                                sc = None
                    feeder.feed(1)
                if sc is not None:
                    drive(sc)

            # 1-deep software pipeline over all 16 (batch, query-block) units:
            # pv(u) runs interleaved with scores(u+1).
            marks = {}
            m0 = feeder.add(*phase1_gens(0))
            feeder.drain_to(m0)
            load_wp()
            marks[1] = feeder.add(*phase1_gens(1))

            scs = {0: scores_gen(0, 0)}
            drive(scs[0])
            for u in range(16):
                b, i = divmod(u, 4)
                nu = u + 1
                if nu < 16:
                    nb, ni = divmod(nu, 4)
                    if ni == 0:
                        feeder.drain_to(marks[nb])   # qT/kT/vaug of batch nb emitted
                    scs[nu] = scores_gen(nb, ni)
                run_pair(pv_gen(b, i), scs.pop(nu, None))
                if i == 3:
                    a2a(b)
                    if b == 0:
                        feeder.add(rvt_gen(0), projmm_gen(0))
                    elif b in (1, 2):
                        feeder.add(rvt_gen(b))
                    if b <= 1:
                        marks[b + 2] = feeder.add(*phase1_gens(b + 2))
                    if b == 3:
                        feeder.add(rvt_gen(3), projmm_gen(1), projmm_gen(2), projmm_gen(3))
            feeder.drain()

    nc.finalize()
    return nc


def _prep_inputs(x, W_attn, b_attn, W_proj, b_proj):
    x = np.asarray(x, dtype=np.float32)
    W_attn = np.asarray(W_attn, dtype=np.float32)
    W_proj = np.asarray(W_proj, dtype=np.float32)

    xT = np.ascontiguousarray(x.reshape(BT, C).T).astype(BF)   # [C, BT]
    wpT = np.ascontiguousarray(W_proj.T).astype(BF)            # [C, C]

    s = np.arange(128)[:, None]
    t = np.arange(128)[None, :]
    ntri = np.where(t >= s, 0.0, -1e9).astype(np.float32)  # valid: key <= query
    ident = np.eye(128, dtype=BF)

    in_maps = []
    for k in range(N_CORES):
        r0 = k * HPC * D                                   # 128*k
        wq_k = np.ascontiguousarray(W_attn[r0 : r0 + 128, :].T).astype(BF)
        wk_k = np.ascontiguousarray(W_attn[C + r0 : C + r0 + 128, :].T).astype(BF)
        wv_k = np.ascontiguousarray(W_attn[2 * C + r0 : 2 * C + r0 + 128, :].T).astype(BF)
        in_maps.append(
            {
                "xT": xT,
                "wq": wq_k,
                "wk": wk_k,
                "wv": wv_k,
                "wp": wpT,
                "ntri": ntri,
                "ident": ident,
            }
        )
    return in_maps


def kernel(x, W_attn, b_attn, W_proj, b_proj):
    global _PROGRAM, LAST_RESULTS
    if _PROGRAM is None:
        _PROGRAM = _build_program()
    nc = _PROGRAM

    in_maps = _prep_inputs(x, W_attn, b_attn, W_proj, b_proj)
    res = run_bass_kernel_spmd(nc, in_maps, list(range(N_CORES)), **RUN_KWARGS)
    LAST_RESULTS = res

    out = np.empty((B, T, C), dtype=np.float32)
    for k in range(N_CORES):
        ok = res.results[k]["out"]                         # [TOKS, C]
        for b in range(B):
            out[b, k * TPB : (k + 1) * TPB, :] = ok[b * TPB : (b + 1) * TPB, :]
    return out


# revision 21
# speedup vs baseline: 1.0617x; 1.0617x over previous
"""Causal self-attention (B=4, T=2048, C=1024, H=16) on 8 TRN2 NeuronCores.

Sharding:
  - QKV + attention: tensor-parallel over heads (2 heads/core, all batches).
  - Output projection: data-parallel over tokens (256 tokens/core/batch),
    connected by one AllToAll per batch (bf16 payload).

v2 layout (all matmul operands bf16, f32 PSUM accumulate):
  - host passes xT = x^T [C, BT] bf16; per-core W_attn q/k/v slices
    transposed [C, 128] bf16; W_proj^T [C, C] bf16 replicated.
  - QKV computes qkvT [dim, tokens] per 512-token block; q^T/k^T persist in
    SBUF [128, BT]; v^T staged per head as vt [64, T] bf16.
  - V^T staged as one [128, T] bf16 tile per batch (both heads stacked);
    XBAR dma-transposes turn each 128-key block into V tiles [keys, 128]
    (head dims side by side); softmax sums accumulate via a separate
    ones-column matmul (1 moving row).
  - scores in S^T layout [keys, queries] per 128-key tile, both heads in one
    [128, 1024] PSUM tile; exp on ScalarE with 1/8 folded into the scale;
    causal = block-range trimming + additive tri-mask on diagonal blocks.
  - P@V flipped: out y_aug [128 queries, 65] PSUM, lhsT = P^T tile,
    rhs = V_aug -> moving dim is 65 (not 512).  Softmax sums ride along as
    column 64 via the V_aug ones column.
  - division: DVE reciprocal of the two sums + per-partition
    tensor_scalar_mul -> ydiv [queries, 128 dims] bf16, DMA'd token-major
    into the A2A send buffer.
  - after the A2A each core XBAR dma-transposes the whole recv buffer
    [2048, 128] -> y^T [128, 2048] in one instruction; projection lhsT
    slices come straight out of it.
"""

import sys

sys.path.insert(0, "/opt/trn_rl_repo")

import numpy as np
import ml_dtypes

import concourse.bass as bass
import concourse.bacc as bacc
import concourse.mybir as mybir
import concourse.tile as tile
from concourse.bass_utils import run_bass_kernel_spmd

N_CORES = 8
B, T, C = 4, 2048, 1024
H, D = 16, 64
HPC = H // N_CORES          # heads per core = 2
BT = B * T                  # 8192 flattened tokens
QB = 512                    # query block
SB = 128                    # key tile
NQB = T // QB               # 4 query blocks per batch
NSB = T // SB               # 16 key tiles per batch
TOKS = BT // N_CORES        # 1024 output tokens per core
TPB = 256                   # tokens per (core, batch)

F32 = mybir.dt.float32
BF16 = mybir.dt.bfloat16
EXP = mybir.ActivationFunctionType.Exp
BF = ml_dtypes.bfloat16

RUN_KWARGS: dict = {}
LAST_RESULTS = None

_PROGRAM = None


def _build_program():
    nc = bacc.Bacc(num_devices=N_CORES)

    xT = nc.declare_dram_parameter("xT", [C, BT], BF16, isOutput=False)
    wq = nc.declare_dram_parameter("wq", [C, 128], BF16, isOutput=False)
    wk = nc.declare_dram_parameter("wk", [C, 128], BF16, isOutput=False)
    wv = nc.declare_dram_parameter("wv", [C, 128], BF16, isOutput=False)
    wp = nc.declare_dram_parameter("wp", [C, C], BF16, isOutput=False)
    ntri = nc.declare_dram_parameter("ntri", [128, 128], F32, isOutput=False)
    out_ext = nc.declare_dram_parameter("out", [TOKS, C], F32, isOutput=True)

    # A2A bounce buffers, token-major: row (m*256 + t) = token t of dest
    # core m, 128 cols = this core's head dims.
    sends = [nc.dram_tensor(f"send{b}", [N_CORES * TPB, 128], BF16) for b in range(B)]
    recvs = [nc.dram_tensor(f"recv{b}", [N_CORES * TPB, 128], BF16) for b in range(B)]

    with tile.TileContext(nc) as tc:
        with (
            tc.tile_pool(name="const", bufs=1) as constp,
            tc.tile_pool(name="wgt", bufs=1) as wgtp,
            tc.tile_pool(name="qk", bufs=1) as qkp,
            tc.tile_pool(name="vt", bufs=2) as vtp,
            tc.tile_pool(name="vpk", bufs=2) as vpkp,
            tc.tile_pool(name="vaug", bufs=4) as vaugp,
            tc.tile_pool(name="xt", bufs=4) as xtp,
            tc.tile_pool(name="pp", bufs=20) as ptp,
            tc.tile_pool(name="ydiv", bufs=4) as ydp,
            tc.tile_pool(name="rc", bufs=8) as rcp,
            tc.tile_pool(name="rvt", bufs=2) as rvtp,
            tc.tile_pool(name="ob", bufs=2) as obp,
            tc.tile_pool(name="ps", bufs=2, space="PSUM") as psp,       # qkv/proj
            tc.tile_pool(name="sps", bufs=2, space="PSUM") as sps,      # S^T [128,1024]
            tc.tile_pool(name="yaps", bufs=2, space="PSUM") as yaps,    # y_aug per head
        ):
            # ---------------- constants + attention weights ----------------
            wq_s = wgtp.tile([128, 8 * 128], BF16, tag="wq")
            wk_s = wgtp.tile([128, 8 * 128], BF16, tag="wk")
            wv_s = wgtp.tile([128, 8 * 128], BF16, tag="wv")
            for dst, src, eng in (
                (wq_s, wq, nc.gpsimd),
                (wk_s, wk, nc.scalar),
                (wv_s, wv, nc.scalar),
            ):
                eng.dma_start(
                    out=dst[:].rearrange("p (c d) -> p c d", c=8),
                    in_=src[:].rearrange("(c p) d -> p c d", p=128),
                )
            ntri_s = constp.tile([128, 128], F32, tag="ntri")
            nc.scalar.dma_start(out=ntri_s[:], in_=ntri[:])
            # wp_s is loaded lazily (emitted after phase1(0)) so it doesn't
            # delay the first xt loads.
            wp_s = wgtp.tile([128, 8 * 1024], BF16, tag="wp")

            def load_wp():
                nc.sync.dma_start(
                    out=wp_s[:].rearrange("p (c d) -> p c d", c=8),
                    in_=wp[:].rearrange("(c p) d -> p c d", p=128),
                )

            qT = qkp.tile([128, BT], BF16, tag="qT")
            kT = qkp.tile([128, BT], BF16, tag="kT")

            vaug = {}   # (b, h) -> [128, NSB*65] bf16 V_aug (ones col at 64)
            vt_hb = {}  # b -> [128, T] bf16 (v^T, both heads stacked)

            def qkv_gen(b, tb):
                """QKV for one 512-token block (generator: yields between
                small PE chunks so attention can interleave)."""
                if tb == 0:
                    vt_hb[b] = vtp.tile([128, T], BF16, tag="vt", name=f"vt{b}")
                base = b * T + tb * QB
                xt = xtp.tile([128, 8 * QB], BF16, tag="xt")
                xt4 = xt[:].rearrange("p (c t) -> p c t", c=8)
                src4 = xT[:].rearrange("(c p) t -> p c t", p=128)[
                    :, :, tb * QB + b * T : tb * QB + b * T + QB
                ]
                # all xt loads on the sync queue: the Pool queue belongs to
                # the collectives, so QKV prefetch never blocks behind one
                nc.sync.dma_start(out=xt4, in_=src4)
                yield
                pq = psp.tile([128, QB], F32, tag="ps")
                pk = psp.tile([128, QB], F32, tag="ps")
                for c in range(8):
                    st = dict(start=(c == 0), stop=(c == 7))
                    xc = xt[:, c * QB : (c + 1) * QB]
                    nc.tensor.matmul(pq[:], wq_s[:, c * 128 : (c + 1) * 128], xc, **st)
                    nc.tensor.matmul(pk[:], wk_s[:, c * 128 : (c + 1) * 128], xc, **st)
                    yield
                nc.vector.tensor_copy(qT[:, base : base + QB], pq[:])
                nc.vector.tensor_copy(kT[:, base : base + QB], pk[:])
                yield
                pv = psp.tile([128, QB], F32, tag="ps")
                for c in range(8):
                    nc.tensor.matmul(
                        pv[:], wv_s[:, c * 128 : (c + 1) * 128], xt[:, c * QB : (c + 1) * QB],
                        start=(c == 0), stop=(c == 7),
                    )
                    if c % 2 == 1:
                        yield
                tloc = tb * QB
                nc.vector.tensor_copy(vt_hb[b][:, tloc : tloc + QB], pv[:])
                yield

            def vaug_gen(b):
                """Per-key-block XBAR dma-transposes of v^T [128, T] into
                packed V tiles [keys, 128] (head dims side by side), then a
                strided DVE copy per head into V_aug [keys, 65-blocks] whose
                ones column (from the initial memset) gives the softmax sums
                in the same matmul accumulation group as the dims."""
                vp = vpkp.tile([128, NSB * 128], BF16, tag="vpk", name=f"vp{b}")
                for j in range(NSB):
                    nc.sync.dma_start_transpose(
                        vp[:, j * 128 : (j + 1) * 128],
                        vt_hb[b][:, j * SB : (j + 1) * SB],
                    )
                    if j % 4 == 3:
                        yield
                for h in range(HPC):
                    va = vaugp.tile([128, NSB * 65], BF16, tag="vaug", name=f"va{b}_{h}")
                    vaug[(b, h)] = va
                    nc.vector.memset(va[:], 1.0)
                    nc.vector.tensor_copy(
                        va[:].rearrange("p (j c) -> p j c", c=65)[:, :, 0:64],
                        vp[:].rearrange("p (j c) -> p j c", c=128)[:, :, h * 64 : h * 64 + 64],
                    )
                    yield

            pt_store = {}  # (b, i) -> list of pt3 views

            def scores_gen(b, i):
                """Scores + exp for all key tiles of one query block."""
                jmax = 4 * (i + 1)
                pt3s = []
                pt_store[(b, i)] = pt3s
                for j in range(jmax):
                    diag = j >= 4 * i
                    r = SB * j - QB * i if diag else 0
                    sp = sps.tile([128, 2 * QB], F32, tag="sp")
                    sp3 = sp[:].rearrange("p (h t) -> p h t", h=2)
                    for h in range(HPC):
                        nc.tensor.matmul(
                            sp3[:, h, r:QB],
                            kT[h * 64 : (h + 1) * 64, b * T + j * SB : b * T + (j + 1) * SB],
                            qT[h * 64 : (h + 1) * 64, b * T + i * QB + r : b * T + (i + 1) * QB],
                            start=True, stop=True,
                        )
                        if diag:
                            nc.vector.tensor_add(
                                sp3[:, h, r : r + 128], sp3[:, h, r : r + 128], ntri_s[:]
                            )
                    pt = ptp.tile([128, 2 * QB], BF16, tag="P")
                    pt3 = pt[:].rearrange("p (h t) -> p h t", h=2)
                    nc.scalar.activation(pt3[:, :, r:QB], sp3[:, :, r:QB], EXP, scale=0.125)
                    pt3s.append(pt3)
                    yield

            def pv_gen(b, i):
                """P@V + softmax division + A2A-send for one query block.
                One PSUM bank per head: accumulation groups in a bank must be
                strictly sequential (sim zero-region semantics)."""
                pt3s = pt_store[(b, i)]
                yah = [yaps.tile([128, 512], F32, tag="ya", name=f"ya{h}") for h in range(HPC)]
                ydiv = None
                for qc in range(4):
                    pi = qc % 2
                    if pi == 0:
                        ydiv = ydp.tile([128, 2 * 128], BF16, tag="ydiv")
                    njt = 4 * i + qc + 1
                    for h in range(HPC):
                        for j in range(njt):
                            nc.tensor.matmul(
                                yah[h][:, 0:65],
                                pt3s[j][:, h, qc * 128 : (qc + 1) * 128],
                                vaug[(b, h)][:, j * 65 : j * 65 + 65],
                                start=(j == 0), stop=(j == njt - 1),
                            )
                        yield
                    for h in range(HPC):
                        rc = rcp.tile([128, 1], F32, tag="rc")
                        nc.vector.reciprocal(rc[:], yah[h][:, 64:65])
                        nc.vector.tensor_scalar_mul(
                            ydiv[:, pi * 128 + h * 64 : pi * 128 + h * 64 + 64],
                            yah[h][:, 0:64],
                            rc[:],
                        )
                    if pi == 1:
                        m = 2 * i + qc // 2
                        nc.sync.dma_start(
                            out=sends[b][m * TPB : (m + 1) * TPB, :].rearrange(
                                "(x q) c -> q x c", x=2
                            ),
                            in_=ydiv[:].rearrange("p (x c) -> p x c", x=2),
                        )
                        yield
                del pt_store[(b, i)]

            def a2a(b):
                nc.gpsimd.collective_compute(
                    "AllToAll",
                    mybir.AluOpType.bypass,
                    replica_groups=[list(range(N_CORES))],
                    ins=[sends[b][:]],
                    outs=[recvs[b][:]],
                )

            rvt_store = {}

            def rvt_gen(b):
                """One XBAR dma-transpose turns the whole recv buffer into
                y^T [128, 2048] (prefetchable as soon as the A2A lands)."""
                rvT = rvtp.tile([128, N_CORES * TPB], BF16, tag="rvT", name=f"rvT{b}")
                rvt_store[b] = rvT
                nc.sync.dma_start_transpose(rvT[:], recvs[b][:])
                yield

            def projmm_gen(b):
                """Projection matmuls for this core's 256 tokens of batch b."""
                rvT = rvt_store[b]
                for tt in range(2):
                    for co in range(2):
                        pj = psp.tile([128, 512], F32, tag="ps", name="pj")
                        for c in range(8):
                            nc.tensor.matmul(
                                pj[:],
                                rvT[:, c * TPB + tt * 128 : c * TPB + tt * 128 + 128],
                                wp_s[:, c * 1024 + co * 512 : c * 1024 + (co + 1) * 512],
                                start=(c == 0), stop=(c == 7),
                            )
                            if c % 2 == 1:
                                yield
                        ob = obp.tile([128, 512], F32, tag="ob")
                        nc.vector.tensor_copy(ob[:], pj[:])
                        row = b * TPB + tt * 128
                        eng = nc.sync if co == 0 else nc.scalar
                        eng.dma_start(
                            out=out_ext[row : row + 128, co * 512 : (co + 1) * 512],
                            in_=ob[:],
                        )
                        yield

            # ---------------- emission schedule ----------------
            from collections import deque

            class Feeder:
                def __init__(self):
                    self.q = deque()
                    self.added = 0
                    self.finished = 0

                def add(self, *gens):
                    self.q.extend(gens)
                    self.added += len(gens)
                    return self.added

                def feed(self, n=1):
                    done = 0
                    while self.q and done < n:
                        try:
                            next(self.q[0])
                            done += 1
                        except StopIteration:
                            self.q.popleft()
                            self.finished += 1
                    return done

                def drain_to(self, mark):
                    while self.finished < mark and self.q:
                        self.feed(64)

                def drain(self):
                    while self.feed(64):
                        pass

            def phase1_gens(b):
                return [qkv_gen(b, tb) for tb in range(4)] + [vaug_gen(b)]

            feeder = Feeder()

            def drive(gen, per_step=1):
                n = 0
                for _ in gen:
                    feeder.feed(per_step)
                    n += 1
                return n

            def run_pair(pv, sc):
                """Drive one unit's P@V to completion while interleaving the
                next unit's scores (2 score steps per P@V step) and feeder
                fillers -- PE gets P@V work while ACT chews on the exps."""
                while True:
                    try:
                        next(pv)
                    except StopIteration:
                        break
                    for _ in range(2):
                        if sc is not None:
                            try:
                                next(sc)
                            except StopIteration:
                                sc = None
                    feeder.feed(1)
                if sc is not None:
                    drive(sc)

            # 1-deep software pipeline over all 16 (batch, query-block) units:
            # pv(u) runs interleaved with scores(u+1).
            marks = {}
            m0 = feeder.add(*phase1_gens(0))
            feeder.drain_to(m0)
            load_wp()
            marks[1] = feeder.add(*phase1_gens(1))

            scs = {0: scores_gen(0, 0)}
            drive(scs[0])
            for u in range(16):
                b, i = divmod(u, 4)
                nu = u + 1
                if nu < 16:
                    nb, ni = divmod(nu, 4)
                    if ni == 0:
                        feeder.drain_to(marks[nb])   # qT/kT/vaug of batch nb emitted
                    scs[nu] = scores_gen(nb, ni)
                run_pair(pv_gen(b, i), scs.pop(nu, None))
                if i == 3:
                    a2a(b)
                    if b == 0:
                        feeder.add(rvt_gen(0), projmm_gen(0))
                    elif b in (1, 2):
                        feeder.add(rvt_gen(b))
                    if b <= 1:
                        marks[b + 2] = feeder.add(*phase1_gens(b + 2))
                    if b == 3:
                        feeder.add(rvt_gen(3), projmm_gen(1), projmm_gen(2), projmm_gen(3))
            feeder.drain()

    nc.finalize()
    return nc


def _prep_inputs(x, W_attn, b_attn, W_proj, b_proj):
    x = np.asarray(x, dtype=np.float32)
    W_attn = np.asarray(W_attn, dtype=np.float32)
    W_proj = np.asarray(W_proj, dtype=np.float32)

    xT = np.ascontiguousarray(x.reshape(BT, C).T).astype(BF)   # [C, BT]
    wpT = np.ascontiguousarray(W_proj.T).astype(BF)            # [C, C]

    s = np.arange(128)[:, None]
    t = np.arange(128)[None, :]
    ntri = np.where(t >= s, 0.0, -1e9).astype(np.float32)  # valid: key <= query

    in_maps = []
    for k in range(N_CORES):
        r0 = k * HPC * D                                   # 128*k
        wq_k = np.ascontiguousarray(W_attn[r0 : r0 + 128, :].T).astype(BF)
        wk_k = np.ascontiguousarray(W_attn[C + r0 : C + r0 + 128, :].T).astype(BF)
        wv_k = np.ascontiguousarray(W_attn[2 * C + r0 : 2 * C + r0 + 128, :].T).astype(BF)
        in_maps.append(
            {
                "xT": xT,
                "wq": wq_k,
                "wk": wk_k,
                "wv": wv_k,
                "wp": wpT,
                "ntri": ntri,
            }
        )
    return in_maps


def kernel(x, W_attn, b_attn, W_proj, b_proj):
    global _PROGRAM, LAST_RESULTS
    if _PROGRAM is None:
        _PROGRAM = _build_program()
    nc = _PROGRAM

    in_maps = _prep_inputs(x, W_attn, b_attn, W_proj, b_proj)
    res = run_bass_kernel_spmd(nc, in_maps, list(range(N_CORES)), **RUN_KWARGS)
    LAST_RESULTS = res

    out = np.empty((B, T, C), dtype=np.float32)
    for k in range(N_CORES):
        ok = res.results[k]["out"]                         # [TOKS, C]
        for b in range(B):
            out[b, k * TPB : (k + 1) * TPB, :] = ok[b * TPB : (b + 1) * TPB, :]
    return out


# revision 22
# speedup vs baseline: 1.1198x; 1.0548x over previous
"""Causal self-attention (B=4, T=2048, C=1024, H=16) on 8 TRN2 NeuronCores.

Sharding:
  - QKV + attention: tensor-parallel over heads (2 heads/core, all batches).
  - Output projection: data-parallel over tokens (256 tokens/core/batch),
    connected by one AllToAll per batch (bf16 payload).

v2 layout (all matmul operands bf16, f32 PSUM accumulate):
  - host passes xT = x^T [C, BT] bf16; per-core W_attn q/k/v slices
    transposed [C, 128] bf16; W_proj^T [C, C] bf16 replicated.
  - QKV computes qkvT [dim, tokens] per 512-token block; q^T/k^T persist in
    SBUF [128, BT]; v^T staged per head as vt [64, T] bf16.
  - V^T staged as one [128, T] bf16 tile per batch (both heads stacked);
    XBAR dma-transposes turn each 128-key block into V tiles [keys, 128]
    (head dims side by side); softmax sums accumulate via a separate
    ones-column matmul (1 moving row).
  - scores in S^T layout [keys, queries] per 128-key tile, both heads in one
    [128, 1024] PSUM tile; exp on ScalarE with 1/8 folded into the scale;
    causal = block-range trimming + additive tri-mask on diagonal blocks.
  - P@V flipped: out y_aug [128 queries, 65] PSUM, lhsT = P^T tile,
    rhs = V_aug -> moving dim is 65 (not 512).  Softmax sums ride along as
    column 64 via the V_aug ones column.
  - division: DVE reciprocal of the two sums + per-partition
    tensor_scalar_mul -> ydiv [queries, 128 dims] bf16, DMA'd token-major
    into the A2A send buffer.
  - after the A2A each core XBAR dma-transposes the whole recv buffer
    [2048, 128] -> y^T [128, 2048] in one instruction; projection lhsT
    slices come straight out of it.
"""

import sys

sys.path.insert(0, "/opt/trn_rl_repo")

import numpy as np
import ml_dtypes

import concourse.bass as bass
import concourse.bacc as bacc
import concourse.mybir as mybir
import concourse.tile as tile
from concourse.bass_utils import run_bass_kernel_spmd

N_CORES = 8
B, T, C = 4, 2048, 1024
H, D = 16, 64
HPC = H // N_CORES          # heads per core = 2
BT = B * T                  # 8192 flattened tokens
QB = 512                    # query block
SB = 128                    # key tile
NQB = T // QB               # 4 query blocks per batch
NSB = T // SB               # 16 key tiles per batch
TOKS = BT // N_CORES        # 1024 output tokens per core
TPB = 256                   # tokens per (core, batch)

F32 = mybir.dt.float32
BF16 = mybir.dt.bfloat16
EXP = mybir.ActivationFunctionType.Exp
BF = ml_dtypes.bfloat16

RUN_KWARGS: dict = {}
LAST_RESULTS = None

_PROGRAM = None


def _build_program():
    nc = bacc.Bacc(num_devices=N_CORES)

    xT = nc.declare_dram_parameter("xT", [C, BT], BF16, isOutput=False)
    wq = nc.declare_dram_parameter("wq", [C, 128], BF16, isOutput=False)
    wk = nc.declare_dram_parameter("wk", [C, 128], BF16, isOutput=False)
    wv = nc.declare_dram_parameter("wv", [C, 128], BF16, isOutput=False)
    wp = nc.declare_dram_parameter("wp", [C, C], BF16, isOutput=False)
    ntri = nc.declare_dram_parameter("ntri", [128, 128], F32, isOutput=False)
    out_ext = nc.declare_dram_parameter("out", [TOKS, C], F32, isOutput=True)

    # A2A bounce buffers, token-major: row (m*256 + t) = token t of dest
    # core m, 128 cols = this core's head dims.
    sends = [nc.dram_tensor(f"send{b}", [N_CORES * TPB, 128], BF16) for b in range(B)]
    recvs = [nc.dram_tensor(f"recv{b}", [N_CORES * TPB, 128], BF16) for b in range(B)]

    with tile.TileContext(nc) as tc:
        with (
            tc.tile_pool(name="const", bufs=1) as constp,
            tc.tile_pool(name="wgt", bufs=1) as wgtp,
            tc.tile_pool(name="qk", bufs=1) as qkp,
            tc.tile_pool(name="vt", bufs=2) as vtp,
            tc.tile_pool(name="vpk", bufs=2) as vpkp,
            tc.tile_pool(name="vaug", bufs=4) as vaugp,
            tc.tile_pool(name="xt", bufs=3) as xtp,
            tc.tile_pool(name="pp", bufs=28) as ptp,
            tc.tile_pool(name="ydiv", bufs=4) as ydp,
            tc.tile_pool(name="rc", bufs=8) as rcp,
            tc.tile_pool(name="rvt", bufs=2) as rvtp,
            tc.tile_pool(name="ob", bufs=2) as obp,
            tc.tile_pool(name="ps", bufs=2, space="PSUM") as psp,       # qkv/proj
            tc.tile_pool(name="sps", bufs=2, space="PSUM") as sps,      # S^T [128,1024]
            tc.tile_pool(name="yaps", bufs=2, space="PSUM") as yaps,    # y_aug per head
        ):
            # ---------------- constants + attention weights ----------------
            wq_s = wgtp.tile([128, 8 * 128], BF16, tag="wq")
            wk_s = wgtp.tile([128, 8 * 128], BF16, tag="wk")
            wv_s = wgtp.tile([128, 8 * 128], BF16, tag="wv")
            for dst, src, eng in (
                (wq_s, wq, nc.gpsimd),
                (wk_s, wk, nc.scalar),
                (wv_s, wv, nc.scalar),
            ):
                eng.dma_start(
                    out=dst[:].rearrange("p (c d) -> p c d", c=8),
                    in_=src[:].rearrange("(c p) d -> p c d", p=128),
                )
            ntri_s = constp.tile([128, 128], F32, tag="ntri")
            nc.scalar.dma_start(out=ntri_s[:], in_=ntri[:])
            # wp_s is loaded lazily (emitted after phase1(0)) so it doesn't
            # delay the first xt loads.
            wp_s = wgtp.tile([128, 8 * 1024], BF16, tag="wp")

            def load_wp():
                nc.sync.dma_start(
                    out=wp_s[:].rearrange("p (c d) -> p c d", c=8),
                    in_=wp[:].rearrange("(c p) d -> p c d", p=128),
                )

            qT = qkp.tile([128, BT], BF16, tag="qT")
            kT = qkp.tile([128, BT], BF16, tag="kT")

            vaug = {}   # (b, h) -> [128, NSB*65] bf16 V_aug (ones col at 64)
            vt_hb = {}  # b -> [128, T] bf16 (v^T, both heads stacked)

            def qkv_gen(b, tb):
                """QKV for one 512-token block (generator: yields between
                small PE chunks so attention can interleave)."""
                if tb == 0:
                    vt_hb[b] = vtp.tile([128, T], BF16, tag="vt", name=f"vt{b}")
                base = b * T + tb * QB
                xt = xtp.tile([128, 8 * QB], BF16, tag="xt")
                xt4 = xt[:].rearrange("p (c t) -> p c t", c=8)
                src4 = xT[:].rearrange("(c p) t -> p c t", p=128)[
                    :, :, tb * QB + b * T : tb * QB + b * T + QB
                ]
                # xt loads on the sync queue: the Pool queue belongs to
                # the collectives, so QKV prefetch never blocks behind one
                if b == 0 and tb == 0:
                    nc.sync.dma_start(out=xt4[:, 0:4, :], in_=src4[:, 0:4, :])
                    nc.gpsimd.dma_start(out=xt4[:, 4:8, :], in_=src4[:, 4:8, :])
                else:
                    nc.sync.dma_start(out=xt4, in_=src4)
                yield
                pq = psp.tile([128, QB], F32, tag="ps")
                pk = psp.tile([128, QB], F32, tag="ps")
                for c in range(8):
                    st = dict(start=(c == 0), stop=(c == 7))
                    xc = xt[:, c * QB : (c + 1) * QB]
                    nc.tensor.matmul(pq[:], wq_s[:, c * 128 : (c + 1) * 128], xc, **st)
                    nc.tensor.matmul(pk[:], wk_s[:, c * 128 : (c + 1) * 128], xc, **st)
                    yield
                nc.vector.tensor_copy(qT[:, base : base + QB], pq[:])
                nc.vector.tensor_copy(kT[:, base : base + QB], pk[:])
                yield
                pv = psp.tile([128, QB], F32, tag="ps")
                for c in range(8):
                    nc.tensor.matmul(
                        pv[:], wv_s[:, c * 128 : (c + 1) * 128], xt[:, c * QB : (c + 1) * QB],
                        start=(c == 0), stop=(c == 7),
                    )
                    if c % 2 == 1:
                        yield
                tloc = tb * QB
                nc.vector.tensor_copy(vt_hb[b][:, tloc : tloc + QB], pv[:])
                yield

            def vaug_gen(b):
                """Per-key-block XBAR dma-transposes of v^T [128, T] into
                packed V tiles [keys, 128] (head dims side by side), then a
                strided DVE copy per head into V_aug [keys, 65-blocks] whose
                ones column (from the initial memset) gives the softmax sums
                in the same matmul accumulation group as the dims."""
                vp = vpkp.tile([128, NSB * 128], BF16, tag="vpk", name=f"vp{b}")
                for j in range(NSB):
                    nc.sync.dma_start_transpose(
                        vp[:, j * 128 : (j + 1) * 128],
                        vt_hb[b][:, j * SB : (j + 1) * SB],
                    )
                    if j % 4 == 3:
                        yield
                for h in range(HPC):
                    va = vaugp.tile([128, NSB * 65], BF16, tag="vaug", name=f"va{b}_{h}")
                    vaug[(b, h)] = va
                    nc.vector.memset(va[:], 1.0)
                    nc.vector.tensor_copy(
                        va[:].rearrange("p (j c) -> p j c", c=65)[:, :, 0:64],
                        vp[:].rearrange("p (j c) -> p j c", c=128)[:, :, h * 64 : h * 64 + 64],
                    )
                    yield

            pt_store = {}  # (b, i) -> list of pt3 views

            def scores_gen(b, i):
                """Scores + exp for all key tiles of one query block."""
                jmax = 4 * (i + 1)
                pt3s = []
                pt_store[(b, i)] = pt3s
                for j in range(jmax):
                    diag = j >= 4 * i
                    r = SB * j - QB * i if diag else 0
                    sp = sps.tile([128, 2 * QB], F32, tag="sp")
                    sp3 = sp[:].rearrange("p (h t) -> p h t", h=2)
                    for h in range(HPC):
                        nc.tensor.matmul(
                            sp3[:, h, r:QB],
                            kT[h * 64 : (h + 1) * 64, b * T + j * SB : b * T + (j + 1) * SB],
                            qT[h * 64 : (h + 1) * 64, b * T + i * QB + r : b * T + (i + 1) * QB],
                            start=True, stop=True,
                        )
                        if diag:
                            nc.vector.tensor_add(
                                sp3[:, h, r : r + 128], sp3[:, h, r : r + 128], ntri_s[:]
                            )
                    pt = ptp.tile([128, 2 * QB], BF16, tag="P")
                    pt3 = pt[:].rearrange("p (h t) -> p h t", h=2)
                    nc.scalar.activation(pt3[:, :, r:QB], sp3[:, :, r:QB], EXP, scale=0.125)
                    pt3s.append(pt3)
                    yield

            def pv_gen(b, i):
                """P@V + softmax division + A2A-send for one query block.
                One PSUM bank per head: accumulation groups in a bank must be
                strictly sequential (sim zero-region semantics)."""
                pt3s = pt_store[(b, i)]
                yah = [yaps.tile([128, 512], F32, tag="ya", name=f"ya{h}") for h in range(HPC)]
                ydiv = None
                for qc in range(4):
                    pi = qc % 2
                    if pi == 0:
                        ydiv = ydp.tile([128, 2 * 128], BF16, tag="ydiv")
                    njt = 4 * i + qc + 1
                    for h in range(HPC):
                        for j in range(njt):
                            nc.tensor.matmul(
                                yah[h][:, 0:65],
                                pt3s[j][:, h, qc * 128 : (qc + 1) * 128],
                                vaug[(b, h)][:, j * 65 : j * 65 + 65],
                                start=(j == 0), stop=(j == njt - 1),
                            )
                        yield
                    for h in range(HPC):
                        rc = rcp.tile([128, 1], F32, tag="rc")
                        nc.vector.reciprocal(rc[:], yah[h][:, 64:65])
                        nc.vector.tensor_scalar_mul(
                            ydiv[:, pi * 128 + h * 64 : pi * 128 + h * 64 + 64],
                            yah[h][:, 0:64],
                            rc[:],
                        )
                    if pi == 1:
                        m = 2 * i + qc // 2
                        nc.sync.dma_start(
                            out=sends[b][m * TPB : (m + 1) * TPB, :].rearrange(
                                "(x q) c -> q x c", x=2
                            ),
                            in_=ydiv[:].rearrange("p (x c) -> p x c", x=2),
                        )
                        yield
                del pt_store[(b, i)]

            def a2a(b):
                nc.gpsimd.collective_compute(
                    "AllToAll",
                    mybir.AluOpType.bypass,
                    replica_groups=[list(range(N_CORES))],
                    ins=[sends[b][:]],
                    outs=[recvs[b][:]],
                )

            rvt_store = {}

            def rvt_gen(b):
                """One XBAR dma-transpose turns the whole recv buffer into
                y^T [128, 2048] (prefetchable as soon as the A2A lands)."""
                rvT = rvtp.tile([128, N_CORES * TPB], BF16, tag="rvT", name=f"rvT{b}")
                rvt_store[b] = rvT
                nc.sync.dma_start_transpose(rvT[:], recvs[b][:])
                yield

            def projmm_gen(b):
                """Projection matmuls for this core's 256 tokens of batch b."""
                rvT = rvt_store[b]
                for tt in range(2):
                    for co in range(2):
                        pj = psp.tile([128, 512], F32, tag="ps", name="pj")
                        for c in range(8):
                            nc.tensor.matmul(
                                pj[:],
                                rvT[:, c * TPB + tt * 128 : c * TPB + tt * 128 + 128],
                                wp_s[:, c * 1024 + co * 512 : c * 1024 + (co + 1) * 512],
                                start=(c == 0), stop=(c == 7),
                            )
                            if c % 2 == 1:
                                yield
                        ob = obp.tile([128, 512], F32, tag="ob")
                        nc.vector.tensor_copy(ob[:], pj[:])
                        row = b * TPB + tt * 128
                        eng = nc.sync if co == 0 else nc.scalar
                        eng.dma_start(
                            out=out_ext[row : row + 128, co * 512 : (co + 1) * 512],
                            in_=ob[:],
                        )
                        yield

            # ---------------- emission schedule ----------------
            from collections import deque

            class Feeder:
                def __init__(self):
                    self.q = deque()
                    self.added = 0
                    self.finished = 0

                def add(self, *gens):
                    self.q.extend(gens)
                    self.added += len(gens)
                    return self.added

                def feed(self, n=1):
                    done = 0
                    while self.q and done < n:
                        try:
                            next(self.q[0])
                            done += 1
                        except StopIteration:
                            self.q.popleft()
                            self.finished += 1
                    return done

                def drain_to(self, mark):
                    while self.finished < mark and self.q:
                        self.feed(64)

                def drain(self):
                    while self.feed(64):
                        pass

            def phase1_gens(b):
                return [qkv_gen(b, tb) for tb in range(4)] + [vaug_gen(b)]

            feeder = Feeder()

            def drive(gen, per_step=1):
                n = 0
                for _ in gen:
                    feeder.feed(per_step)
                    n += 1
                return n

            sc_fifo = deque()

            def pump_scores(n):
                while n and sc_fifo:
                    try:
                        next(sc_fifo[0])
                        n -= 1
                    except StopIteration:
                        sc_fifo.popleft()

            def run_pv(pv):
                """Drive one unit's P@V while pumping the scores FIFO (3
                score steps per P@V step) and feeder fillers -- PE gets P@V
                work while ACT chews on the exps, and scores emission runs
                ahead as deep as the pt pool allows."""
                while True:
                    try:
                        next(pv)
                    except StopIteration:
                        break
                    pump_scores(3)
                    feeder.feed(1)

            # deep software pipeline over all 16 (batch, query-block) units
            marks = {}
            m0 = feeder.add(*phase1_gens(0))
            feeder.drain_to(m0)
            load_wp()
            marks[1] = feeder.add(*phase1_gens(1))

            sc_fifo.append(scores_gen(0, 0))
            pump_scores(4)
            for u in range(16):
                b, i = divmod(u, 4)
                nu = u + 1
                if nu < 16:
                    nb, ni = divmod(nu, 4)
                    if ni == 0:
                        feeder.drain_to(marks[nb])   # qT/kT/vaug of batch nb emitted
                    sc_fifo.append(scores_gen(nb, ni))
                run_pv(pv_gen(b, i))
                if i == 3:
                    a2a(b)
                    if b <= 1:
                        marks[b + 2] = feeder.add(*phase1_gens(b + 2))
                    if b == 0:
                        feeder.add(rvt_gen(0), projmm_gen(0))
                    elif b in (1, 2):
                        feeder.add(rvt_gen(b))
                    if b == 3:
                        feeder.add(rvt_gen(3), projmm_gen(1), projmm_gen(2), projmm_gen(3))
            while sc_fifo:
                pump_scores(8)
                feeder.feed(1)
            feeder.drain()

    nc.finalize()
    return nc


def _prep_inputs(x, W_attn, b_attn, W_proj, b_proj):
    x = np.asarray(x, dtype=np.float32)
    W_attn = np.asarray(W_attn, dtype=np.float32)
    W_proj = np.asarray(W_proj, dtype=np.float32)

    xT = np.ascontiguousarray(x.reshape(BT, C).T).astype(BF)   # [C, BT]
    wpT = np.ascontiguousarray(W_proj.T).astype(BF)            # [C, C]

    s = np.arange(128)[:, None]
    t = np.arange(128)[None, :]
    ntri = np.where(t >= s, 0.0, -1e9).astype(np.float32)  # valid: key <= query

    in_maps = []
    for k in range(N_CORES):
        r0 = k * HPC * D                                   # 128*k
        wq_k = np.ascontiguousarray(W_attn[r0 : r0 + 128, :].T).astype(BF)
        wk_k = np.ascontiguousarray(W_attn[C + r0 : C + r0 + 128, :].T).astype(BF)
        wv_k = np.ascontiguousarray(W_attn[2 * C + r0 : 2 * C + r0 + 128, :].T).astype(BF)
        in_maps.append(
            {
                "xT": xT,
                "wq": wq_k,
                "wk": wk_k,
                "wv": wv_k,
                "wp": wpT,
                "ntri": ntri,
            }
        )
    return in_maps


def kernel(x, W_attn, b_attn, W_proj, b_proj):
    global _PROGRAM, LAST_RESULTS
    if _PROGRAM is None:
        _PROGRAM = _build_program()
    nc = _PROGRAM

    in_maps = _prep_inputs(x, W_attn, b_attn, W_proj, b_proj)
    res = run_bass_kernel_spmd(nc, in_maps, list(range(N_CORES)), **RUN_KWARGS)
    LAST_RESULTS = res

    out = np.empty((B, T, C), dtype=np.float32)
    for k in range(N_CORES):
        ok = res.results[k]["out"]                         # [TOKS, C]
        for b in range(B):
            out[b, k * TPB : (k + 1) * TPB, :] = ok[b * TPB : (b + 1) * TPB, :]
    return out
